# revision 16
# baseline (speedup 1.0000x reference)
"""Trainium2 Bass kernel, 8-core time-sharded version.

Time axis is split into 8 slices of 12544 steps (one per core). Each core
solves its slice plus a 256-step leading halo, laid out [128 chunks x 100],
with parallel-in-time Newton: linearize the contraction map, solve the
affine recurrence exactly with the hardware tensor_tensor_scan + a
PE-transpose chunk combine across the 128 partitions. Warm start + 4
iterations reach max rel err ~4e-7 vs the float64 orbit (proto3.py).

Per-core inputs differ only in data (SPMD): each core gets its time slice
plus a 256-step HALO before it, solved from a zero start-state guess — the
contraction (0.857^256 ~ 8e-18) makes the halo-start error vanish before the
kept region begins, so no cross-core exchange is needed at all.
"""
import numpy as np

U1MAX = 221.519
ML = 2.9086
SL = 1.898
SPIN = 365
TRAIN = 80000
B = 100000
GU = 32
PN, LNS = 128, 99
TOT = PN * LNS            # 12672 per core (incl. halo)
KEEP = 12544              # kept per core
HALO = TOT - KEEP         # 128 (0.857^128 ~ 2.5e-9: below fp32 noise)
N_CORES = 8
NEWTON_ITERS = 3
NOBS = TRAIN - SPIN       # 79635
YL = 623                  # y layout [128, 623]; pad = 128*623 - NOBS = 109
YPAD = PN * YL - NOBS

_cache = {}


def _build(consts, gate_consts):
    import concourse.bacc as bacc
    import concourse.mybir as mybir
    from concourse.tile import TileContext
    from concourse.masks import make_identity

    dt = mybir.dt.float32
    Alu = mybir.AluOpType
    AF = mybir.ActivationFunctionType

    alpha = consts["alpha"]; beta = consts["beta"]
    oo1 = consts["oo1"]; ol1 = consts["ol1"]
    K1 = oo1 * alpha
    w2s = consts["w2s"]; b2s = consts["b2s"]
    sgbar = consts["sgbar"]

    nc = bacc.Bacc("TRN2", target_bir_lowering=False, debug=False, num_devices=N_CORES)
    xs_d = nc.dram_tensor("xs", [PN, 2 * LNS], dt, kind="ExternalInput")
    yp_d = nc.dram_tensor("ypad", [PN, YL], dt, kind="ExternalInput")
    outs = {}
    for name in ["h_n", "c_n", "l_n", "lc_n", "gate_oo", "gate_ol", "gate_olc",
                 "gate_f"] + (["bc_n"] if gate_consts else []):
        outs[name] = nc.dram_tensor(name, [PN, LNS], dt, kind="ExternalOutput")
    std_d = nc.dram_tensor("obs_std", [1, 1], dt, kind="ExternalOutput")

    with TileContext(nc) as tc:
        with tc.tile_pool(name="const", bufs=1) as cp, \
             tc.tile_pool(name="main", bufs=1) as mp, \
             tc.tile_pool(name="state", bufs=2) as st, \
             tc.tile_pool(name="scr", bufs=1) as sc, \
             tc.tile_pool(name="po", bufs=2) as po, \
             tc.tile_pool(name="psum", bufs=1, space="PSUM") as pp:

            def big(pool, tag):
                return pool.tile([PN, LNS], dt, tag=tag, name=tag)

            # ---- constants ----
            ones = big(cp, "ones")
            nc.vector.memset(ones[:], 1.0)
            bias_beta = cp.tile([PN, 1], dt, tag="bias_beta")
            nc.vector.memset(bias_beta[:], beta)
            bias_b2 = cp.tile([PN, 1], dt, tag="bias_b2")
            nc.vector.memset(bias_b2[:], b2s)
            ident = cp.tile([PN, PN], dt, tag="ident")
            make_identity(nc, ident[:])
            one11 = cp.tile([1, 1], dt, tag="one11")
            nc.vector.memset(one11[:], 1.0)

            # ---- inputs ----
            xint = mp.tile([PN, 2 * LNS], dt, tag="xint")
            xpair = xint[:].rearrange("p (l two) -> p l two", two=2)
            u1 = big(mp, "u1")
            u2 = big(mp, "u2")
            for p0 in (0, 64):  # split so copies overlap the DMAs
                nc.sync.dma_start(xint[p0:p0 + 64, :], xs_d.ap()[p0:p0 + 64, :])
                nc.vector.tensor_copy(u1[p0:p0 + 64, :], xpair[p0:p0 + 64, :, 0])
                nc.vector.tensor_copy(u2[p0:p0 + 64, :], xpair[p0:p0 + 64, :, 1])

            # ---- obs_std over [128, 623] (last 109 are zero pad) ----
            yb = mp.tile([PN, YL], dt, tag="yb")
            nc.sync.dma_start(yb[:], yp_d.ap())
            # cross-partition reduce/broadcast via PE (GpSimd has ~6us
            # wake-up latency); sqrt of the variance happens on host
            ones_row = cp.tile([1, PN], dt, tag="ones_row")
            nc.vector.memset(ones_row[:], 1.0)
            r1 = mp.tile([PN, 1], dt, tag="r1")
            nc.vector.tensor_reduce(r1[:], yb[:], mybir.AxisListType.X, Alu.add)
            totp = pp.tile([1, 1], dt, tag="totp", name="totp")
            nc.tensor.matmul(totp[:], r1[:], ones[:, 0:1], start=True, stop=True)
            nmean = mp.tile([1, 1], dt, tag="nmean")
            nc.vector.tensor_scalar(nmean[:], totp[:], -1.0 / NOBS, None, Alu.mult)
            nmbp = pp.tile([PN, 1], dt, tag="nmbp", name="nmbp")
            nc.tensor.matmul(nmbp[:], ones_row[:], nmean[:], start=True, stop=True)
            nmb = mp.tile([PN, 1], dt, tag="nmb")
            nc.vector.tensor_copy(nmb[:], nmbp[:])
            nc.vector.tensor_scalar(yb[:], yb[:], nmb[:], None, Alu.add)
            nc.vector.tensor_tensor(yb[:], yb[:], yb[:], Alu.mult)
            nc.vector.tensor_reduce(r1[:], yb[:], mybir.AxisListType.X, Alu.add)
            nc.tensor.matmul(totp[:], r1[:], ones[:, 0:1], start=True, stop=True)
            # subtract pad contribution: pad cells each contribute mean^2
            m2 = mp.tile([1, 1], dt, tag="m2")
            nc.vector.tensor_tensor(m2[:], nmean[:], nmean[:], Alu.mult)
            tot = mp.tile([1, 1], dt, tag="tot")
            nc.vector.scalar_tensor_tensor(tot[:], m2[:], -float(YPAD), totp[:],
                                           Alu.mult, Alu.add)
            nc.vector.tensor_scalar(tot[:], tot[:], 1.0 / (NOBS - 1), None, Alu.mult)
            nc.sync.dma_start(std_d.ap(), tot[:])

            # ---- phase 1 ----
            olpre = big(mp, "olpre")
            nc.scalar.activation(olpre[:], u2[:], AF.Sigmoid, bias=bias_b2[:],
                                 scale=w2s)
            nc.vector.tensor_scalar(olpre[:], olpre[:], ol1, None, Alu.mult)
            olc1 = big(mp, "olc1")
            nc.vector.tensor_scalar(olc1[:], olpre[:], -1.0, 1.0, Alu.mult, Alu.add)

            if gate_consts:
                bc = big(mp, "bc")
                nc.vector.memset(bc[:], 0.0)
                for idx, (bj, wj) in enumerate(gate_consts):
                    bt = cp.tile([PN, 1], dt, tag=f"bj{idx}", name=f"bj{idx}")
                    nc.vector.memset(bt[:], -float(bj))
                    fu = big(sc, "S1")
                    nc.scalar.activation(fu[:], u1[:], AF.Relu,
                                         bias=bt[:], scale=1.0 / U1MAX)
                    nc.vector.scalar_tensor_tensor(bc[:], fu[:], float(wj), bc[:],
                                                   Alu.mult, Alu.add)
                g = big(mp, "g")
                nc.vector.scalar_tensor_tensor(g[:], bc[:], U1MAX, u1[:],
                                               Alu.mult, Alu.add)
                nc.sync.dma_start(outs["bc_n"].ap(), bc[:])
            else:
                g = u1

            xs_row = mp.tile([1, PN], dt, tag="xs_row")
            nc.vector.memset(xs_row[:], 0.0)

            def solve_affine(a_ap, b_ap):
                """Solve x_{t+1} = a_t x_t + b_t (x_0 = 0) exactly via the
                hardware per-partition scan + a PE-transpose chunk combine."""
                Ssc = big(sc, "Ssc")
                nc.vector.tensor_tensor_scan(Ssc[:], a_ap, b_ap, 0.0,
                                             Alu.mult, Alu.add)
                Psc = big(sc, "Psc")
                nc.vector.tensor_tensor_scan(Psc[:], a_ap, ones[:], 1.0,
                                             Alu.mult, Alu.mult)
                tpsP = pp.tile([1, PN], dt, tag="tpsP", name="tpsP", bufs=2)
                nc.tensor.transpose(tpsP[:], Psc[:, LNS - 1:LNS], ident[:])
                tpsS = pp.tile([1, PN], dt, tag="tpsS", name="tpsS", bufs=2)
                nc.tensor.transpose(tpsS[:], Ssc[:, LNS - 1:LNS], ident[:])
                rowP = sc.tile([1, PN], dt, tag="rowP", name="rowP")
                nc.vector.tensor_copy(rowP[:], tpsP[:])
                rowS = sc.tile([1, PN], dt, tag="rowS", name="rowS")
                nc.vector.tensor_copy(rowS[:], tpsS[:])
                l2 = sc.tile([1, PN], dt, tag="l2", name="l2")
                nc.vector.tensor_tensor_scan(l2[:], rowP[:], rowS[:], 0.0,
                                             Alu.mult, Alu.add)
                nc.vector.tensor_copy(xs_row[:, 1:PN], l2[:, 0:PN - 1])
                xs_ps = pp.tile([PN, 1], dt, tag="xs_ps", name="xs_ps", bufs=2)
                nc.tensor.matmul(xs_ps[:], xs_row[:], one11[:],
                                 start=True, stop=True)
                xs = sc.tile([PN, 1], dt, tag="xs", name="xs")
                nc.vector.tensor_copy(xs[:], xs_ps[:])
                Cn = big(st, "C")
                nc.vector.tensor_copy(Cn[:, 0:1], xs[:])
                nc.vector.scalar_tensor_tensor(
                    Cn[:, 1:LNS], Psc[:, 0:LNS - 1], xs[:],
                    Ssc[:, 0:LNS - 1], Alu.mult, Alu.add)
                return Cn

            # ---- init: solve the constant-sigmoid linearization exactly ----
            S1i = big(sc, "S1")
            nc.vector.tensor_scalar(S1i[:], olc1[:], -oo1 * sgbar, None, Alu.add)
            C = solve_affine(S1i[:], g[:])

            def newton_iter(C):
                """One Newton iteration on [128, LNS]."""
                S1 = big(sc, "S1")
                nc.scalar.activation(S1[:], C[:], AF.Sigmoid,
                                     bias=bias_beta[:], scale=alpha)
                S2 = big(sc, "S2")
                nc.vector.tensor_tensor(S2[:], C[:], olpre[:], Alu.mult)
                S3 = big(sc, "S3")
                nc.vector.tensor_tensor(S3[:], S2[:], u2[:], Alu.is_ge)
                nc.vector.tensor_tensor(S2[:], S3[:], u2[:], Alu.mult)      # w
                S4 = big(sc, "S4")
                nc.vector.tensor_tensor(S4[:], S3[:], olpre[:], Alu.mult)   # om
                S5 = big(sc, "S5")
                nc.vector.tensor_tensor(S5[:], S1[:], S1[:], Alu.mult)
                nc.vector.tensor_tensor(S5[:], S1[:], S5[:], Alu.subtract)  # ds
                nc.vector.tensor_tensor(S5[:], S5[:], C[:], Alu.mult)       # e
                nc.vector.scalar_tensor_tensor(S1[:], S1[:], -oo1, olc1[:],
                                               Alu.mult, Alu.add)
                nc.vector.scalar_tensor_tensor(S1[:], S5[:], -K1, S1[:],
                                               Alu.mult, Alu.add)
                nc.vector.tensor_tensor(S1[:], S1[:], S4[:], Alu.add)       # a
                nc.vector.tensor_tensor(S5[:], S5[:], C[:], Alu.mult)       # ec
                nc.vector.tensor_tensor(S2[:], g[:], S2[:], Alu.subtract)   # gw
                nc.vector.scalar_tensor_tensor(S2[:], S5[:], K1, S2[:],
                                               Alu.mult, Alu.add)           # b
                return solve_affine(S1[:], S2[:])

            for _ in range(NEWTON_ITERS):
                C = newton_iter(C)

            # ---- phase 3 ----
            nc.sync.dma_start(outs["c_n"].ap(), C[:])
            nc.sync.dma_start(outs["gate_ol"].ap(), olpre[:])
            sg3 = big(sc, "S1")
            nc.scalar.activation(sg3[:], C[:], AF.Sigmoid,
                                 bias=bias_beta[:], scale=alpha)
            oo = big(po, "oo")
            nc.vector.tensor_scalar(oo[:], sg3[:], oo1, None, Alu.mult)
            nc.sync.dma_start(outs["gate_oo"].ap(), oo[:])
            h = big(po, "t")
            nc.vector.tensor_tensor(h[:], oo[:], C[:], Alu.mult)
            nc.sync.dma_start(outs["h_n"].ap(), h[:])
            l = big(po, "t")
            nc.vector.tensor_tensor(l[:], olpre[:], C[:], Alu.mult)
            nc.sync.dma_start(outs["l_n"].ap(), l[:])
            dgt = big(sc, "S2")
            nc.vector.tensor_scalar(dgt[:], C[:], 0.0, None, Alu.is_gt)
            den = big(sc, "S3")  # (C-1)*dgt + 1 == C if C>0 else 1
            nc.vector.tensor_scalar(den[:], C[:], 1.0, None, Alu.subtract)
            nc.vector.tensor_tensor(den[:], den[:], dgt[:], Alu.mult)
            nc.vector.tensor_scalar(den[:], den[:], 1.0, None, Alu.add)
            nc.vector.reciprocal(den[:], den[:])
            ratio = big(sc, "S4")
            nc.vector.tensor_tensor(ratio[:], u2[:], den[:], Alu.mult)
            nc.vector.tensor_tensor(ratio[:], olpre[:], ratio[:], Alu.subtract)
            nc.vector.tensor_scalar(ratio[:], ratio[:], 0.0, None, Alu.max)
            nc.vector.tensor_tensor(ratio[:], ratio[:], dgt[:], Alu.mult)
            olc = big(po, "olc")
            nc.vector.tensor_tensor(olc[:], olpre[:], ratio[:], Alu.subtract)
            nc.sync.dma_start(outs["gate_olc"].ap(), olc[:])
            lc = big(po, "t")
            nc.vector.tensor_tensor(lc[:], olc[:], C[:], Alu.mult)
            nc.sync.dma_start(outs["lc_n"].ap(), lc[:])
            f = big(po, "t")
            nc.vector.tensor_tensor(f[:], oo[:], olc[:], Alu.add)
            nc.vector.tensor_scalar(f[:], f[:], -1.0, 1.0, Alu.mult, Alu.add)
            nc.sync.dma_start(outs["gate_f"].ap(), f[:])


    nc.compile()
    return nc


def kernel(**inputs):
    from concourse import bass_utils

    x = np.ascontiguousarray(np.asarray(inputs["x"], dtype=np.float32))
    y_obs = np.ascontiguousarray(np.asarray(inputs["y_obs"], dtype=np.float32))
    cmean = float(np.asarray(inputs["cmean"]).reshape(-1)[0])
    cstd = float(np.asarray(inputs["cstd"]).reshape(-1)[0])
    w_r_yom = np.asarray(inputs["weight_r_yom"], np.float32)
    w_r_ylm = np.asarray(inputs["weight_r_ylm"], np.float32)
    w_r_yfm = np.asarray(inputs["weight_r_yfm"], np.float32)
    b0_yom = float(np.asarray(inputs["bias_b0_yom"]).reshape(-1)[0])
    w_b1 = float(np.asarray(inputs["weight_b1_yom"]).reshape(-1)[0])
    b0_ylm = float(np.asarray(inputs["bias_b0_ylm"]).reshape(-1)[0])
    w_b2 = float(np.asarray(inputs["weight_b2_ylm"]).reshape(-1)[0])
    ln_wj = np.asarray(inputs["ln_wj"], np.float32).reshape(GU)
    relu_bj = np.asarray(inputs["relu_bj"], np.float32).reshape(GU)

    eo = np.exp(np.float32(w_r_yom[0, 0]))
    el = np.exp(np.float32(w_r_ylm[0, 0]))
    ef = np.exp(np.float32(w_r_yfm[0, 0]))
    den = np.float32(eo + el + ef)
    oo1 = float(eo / den)
    ol1 = float(el / den)
    alpha = w_b1 / cstd
    beta = b0_yom - cmean * w_b1 / cstd
    w2s = w_b2 / SL
    b2s = b0_ylm - ML * w_b2 / SL
    sgbar = float(1.0 / (1.0 + np.exp(-(alpha * 1.5 + beta))))

    bj = np.maximum(relu_bj, 0.0)
    u1max = float(x[:, 0, 0].max())
    active = [(float(bj[j]), float(ln_wj[j])) for j in range(GU)
              if bj[j] < u1max / U1MAX]

    key = (round(alpha, 9), round(beta, 9), round(oo1, 9), round(ol1, 9),
           round(w2s, 9), round(b2s, 9), tuple(active))
    if key not in _cache:
        consts = dict(alpha=alpha, beta=beta, oo1=oo1, ol1=ol1,
                      w2s=w2s, b2s=b2s, sgbar=sgbar)
        _cache[key] = _build(consts, active)
    nc = _cache[key]

    xflat = x.reshape(-1)  # [2B] interleaved (u1, u2)
    ypad = np.zeros(PN * YL, np.float32)
    ypad[:NOBS] = y_obs[SPIN:TRAIN, 0]
    ypad = ypad.reshape(PN, YL)
    in_maps = []
    for k in range(N_CORES):
        lo = k * KEEP - HALO  # global start (may be negative / past end)
        xsl = np.zeros(2 * TOT, np.float32)
        s0 = max(lo, 0); s1 = min(lo + TOT, B)
        if s1 > s0:
            xsl[2 * (s0 - lo):2 * (s0 - lo) + 2 * (s1 - s0)] = \
                xflat[2 * s0:2 * s1]
        in_maps.append({
            "xs": xsl.reshape(PN, 2 * LNS),
            "ypad": ypad,
        })
    res = bass_utils.run_bass_kernel_spmd(nc, in_maps,
                                          core_ids=list(range(N_CORES)))
    global LAST_RESULTS
    LAST_RESULTS = res

    def full(name):
        parts = [res.results[k][name].reshape(-1)[HALO:] for k in range(N_CORES)]
        return np.ascontiguousarray(
            np.concatenate(parts)[:B].reshape(B, 1).astype(np.float32))

    h_n = full("h_n")
    c_n = full("c_n")
    l_n = full("l_n")
    lc_n = full("lc_n")
    gate_oo = full("gate_oo")
    gate_ol = full("gate_ol")
    gate_olc = full("gate_olc")
    gate_f = full("gate_f")
    bc_n = full("bc_n") if active else np.zeros((B, 1), np.float32)
    obsstd = np.float32(np.sqrt(res.results[0]["obs_std"][0, 0]))
    zeros = np.zeros((B, 1), np.float32)
    obs_std = np.full((B, 1), obsstd, np.float32)
    h_nout = np.concatenate([h_n, obs_std], axis=1)

    return (h_n, c_n, l_n, lc_n, zeros, zeros.copy(), gate_oo, gate_ol,
            gate_olc, gate_f, bc_n, h_nout, obs_std)


LAST_RESULTS = None
